# revision 1
# baseline (speedup 1.0000x reference)
"""ConvexSH ColBERT loss kernel for 8 trn2 NeuronCores.

Shards batch B=128 over 8 cores (16 rows each). Each core sees all NWAY=8
candidates for its rows, so softmax + loss are core-local; the host averages
the 8 partial sums (the "all-reduce mean" of the sharding hint).

Pipeline per core and candidate n (2 MB doc block):
  SWDGE cast-DMA f32->bf16  ->  DVE fused square+row-sum (ssq)
  -> small batched ops for masked inv-norms -> GpSimd per-token normalize
  -> PE transpose [k,d]->[d,k] -> ACT/DVE PSUM evacuation
  -> PE bf16 matmul (4-way column-tiled, full 128-partition PSUM)
  -> DVE reduce_max over k.
Tail: block-ones matmul (partition sums) -> scores [4,32] -> softmax +
ConvexSH loss on-chip -> scalar partial sum.
"""

import sys
from contextlib import ExitStack

import numpy as np

for _p in ("/opt/trn_rl_repo", "/root/.axon_site/_ro/trn_rl_repo"):
    if _p not in sys.path:
        sys.path.append(_p)

import concourse.bacc as bacc
import concourse.tile as tile
from concourse import mybir
from concourse.bass_utils import run_bass_kernel_spmd

AF = mybir.ActivationFunctionType
AX = mybir.AxisListType
ALU = mybir.AluOpType
F32 = mybir.dt.float32
BF16 = mybir.dt.bfloat16

NCORES = 8
B, LQ, LD, D, NWAY = 128, 32, 256, 128, 8
BS = B // NCORES  # 16 batch rows per core
NG = BS // 4      # 4 groups of 4 rows (PSUM partition packing)
ALPHA, GAMMA, EPS = 0.2, 2.0, 1e-12

USE_BF16 = True   # cast doc/query to bf16 in-flight; sim matmul in bf16

TRACE = False
LAST_RESULTS = None


def _build():
    # Bacc: its finalize() runs move_matmul_waits_to_ldweights +
    # generate_event_semaphores, required by this walrus build's
    # one-sync-wait-per-instruction limit.
    nc = bacc.Bacc("TRN2", target_bir_lowering=False, detect_race_conditions=False)
    DT = BF16 if USE_BF16 else F32

    q_d = nc.dram_tensor("q", [BS, LQ, D], F32, kind="ExternalInput")
    doc_d = nc.dram_tensor("doc", [NWAY, BS, LD, D], F32, kind="ExternalInput")
    mask_d = nc.dram_tensor("mask", [NWAY, BS, LD], F32, kind="ExternalInput")
    lab_d = nc.dram_tensor("lab", [BS, 3 * NWAY], F32, kind="ExternalInput")
    eye_d = nc.dram_tensor("eye", [128, 128], F32, kind="ExternalInput")
    y_d = nc.dram_tensor("y", [1, 1], F32, kind="ExternalOutput")

    def cast_dma(out, in_):
        if USE_BF16:
            nc.gpsimd.dma_start(out=out, in_=in_)  # SWDGE: casts f32->bf16
        else:
            nc.sync.dma_start(out=out, in_=in_)

    with tile.TileContext(nc) as tc, ExitStack() as ctx:
        singles = ctx.enter_context(tc.tile_pool(name="singles", bufs=1))
        dpool = ctx.enter_context(tc.tile_pool(name="dpool", bufs=2))
        sqpool = ctx.enter_context(tc.tile_pool(name="sqpool", bufs=4))
        npool = ctx.enter_context(tc.tile_pool(name="npool", bufs=2))
        dtpool = ctx.enter_context(tc.tile_pool(name="dtpool", bufs=4))
        psT = ctx.enter_context(tc.tile_pool(name="psT", bufs=2, space="PSUM"))
        psM = ctx.enter_context(tc.tile_pool(name="psM", bufs=1, space="PSUM"))
        psMM = ctx.enter_context(tc.tile_pool(name="psMM", bufs=2, space="PSUM"))
        psS = ctx.enter_context(tc.tile_pool(name="psS", bufs=1, space="PSUM"))

        # ---- constants / setup -------------------------------------------
        eye_f = singles.tile([128, 128], F32)
        nc.sync.dma_start(out=eye_f, in_=eye_d[:, :])
        if USE_BF16:
            eye_sb = singles.tile([128, 128], BF16)
            nc.vector.tensor_copy(eye_sb, eye_f)
        else:
            eye_sb = eye_f

        blockones = singles.tile([128, NG], F32)
        nc.vector.memset(blockones, 0.0)
        for m in range(4):
            nc.vector.memset(blockones[m * 32:(m + 1) * 32, m:m + 1], 1.0)
        ones4 = singles.tile([4, 1], F32)
        nc.vector.memset(ones4, 1.0)

        # labels, partition = b%4, free = (g, col)
        lab_sb = singles.tile([4, NG, 3 * NWAY], F32)
        nc.sync.dma_start(out=lab_sb, in_=lab_d.rearrange("(g m) c -> m g c", m=4))

        # ---- query: ssq + transpose --------------------------------------
        # partition = (b%4)*32 + q, tiles t = b//4 (= group g)
        q_nat = singles.tile([128, NG, D], DT)
        cast_dma(q_nat, q_d.rearrange("(t r) q d -> (r q) t d", r=4))

        ssq_q = singles.tile([128, NG], F32)
        for t in range(NG):
            sq_t = sqpool.tile([128, D], DT, tag="sq")
            nc.vector.scalar_tensor_tensor(
                out=sq_t, in0=q_nat[:, t, :], scalar=1.0, in1=q_nat[:, t, :],
                op0=ALU.mult, op1=ALU.mult,
                accum_out=ssq_q[:, t:t + 1])
        invq = singles.tile([128, NG], F32)
        nc.scalar.activation(out=invq, in_=ssq_q, func=AF.Sqrt)
        nc.vector.tensor_scalar_max(invq, invq, EPS)
        nc.vector.reciprocal(invq, invq)

        qT = singles.tile([128, BS * LQ], DT)  # [d, token], token = b*32+q
        for pair in range(2):
            ps = psT.tile([128, 256], DT, tag="psT")
            for h in range(2):
                t = pair * 2 + h
                nc.tensor.transpose(ps[:, h * 128:(h + 1) * 128], q_nat[:, t, :], eye_sb)
            nc.vector.tensor_copy(qT[:, pair * 256:(pair + 1) * 256], ps)

        # ---- masks: transpose to [k, (n,b)] (f32 path, setup-only) -------
        mask_nat = singles.tile([128, LD], F32)  # partition = n*16+b
        nc.sync.dma_start(out=mask_nat, in_=mask_d.rearrange("n b k -> (n b) k"))
        maskT = singles.tile([128, 2, 128], F32)  # [k%128, h, n*16+b]
        psm = psM.tile([128, 256], F32, tag="psM")
        for h in range(2):
            nc.tensor.transpose(psm[:, h * 128:(h + 1) * 128], mask_nat[:, h * 128:(h + 1) * 128], eye_f)
        nc.vector.tensor_copy(maskT.rearrange("p h k -> p (h k)"), psm)

        # maxs[p, g*8+n]: p = (b%4)*32 + q
        maxs = singles.tile([128, NG * NWAY], F32)

        # ---- main loop over candidates n ---------------------------------
        for n in range(NWAY):
            # doc block, partition = within-half k, tiles j = h*16 + b
            dn = dpool.tile([128, 2 * BS, D], DT, tag="dn")
            dsrc = doc_d[n].rearrange("b (h p) d -> p h b d", p=128)
            for h in range(2):
                cast_dma(dn[:, h * BS:(h + 1) * BS, :], dsrc[:, h])

            # sum of squares per token: one big ACT square pass (fixed cost
            # amortized over FD=4096) + one big 3D DVE reduce
            sq_n = sqpool.tile([128, 2 * BS, D], DT, tag="sq")
            nc.scalar.activation(out=sq_n.rearrange("p j d -> p (j d)"),
                                 in_=dn.rearrange("p j d -> p (j d)"),
                                 func=AF.Square)
            ssq_n = npool.tile([128, 2 * BS], F32, tag="ssq")
            nc.vector.reduce_sum(out=ssq_n, in_=sq_n, axis=AX.X)

            # scale = m / max(m * sqrt(ssq), eps); cols j = h*16+b contiguous per h
            scale = npool.tile([128, 2 * BS], F32, tag="scale")
            nc.scalar.activation(out=scale, in_=ssq_n, func=AF.Sqrt)
            for h in range(2):
                hs = slice(h * BS, (h + 1) * BS)
                mh = maskT[:, h, n * BS:(n + 1) * BS]
                nc.vector.tensor_mul(scale[:, hs], scale[:, hs], mh)
            nc.vector.tensor_scalar_max(scale, scale, EPS)
            nc.vector.reciprocal(scale, scale)
            for h in range(2):
                hs = slice(h * BS, (h + 1) * BS)
                mh = maskT[:, h, n * BS:(n + 1) * BS]
                nc.vector.tensor_mul(scale[:, hs], scale[:, hs], mh)

            # normalize in place; ~1/3 of the tiles go to ACT for balance
            for j in range(2 * BS):
                if j % 3 == 2:
                    nc.scalar.mul(dn[:, j, :], dn[:, j, :], scale[:, j:j + 1])
                else:
                    nc.vector.tensor_scalar_mul(dn[:, j, :], dn[:, j, :], scale[:, j:j + 1])

            # per group of 4 rows: transpose (8 tiles into one PSUM bank),
            # one big evacuation, 4 col-tiled matmuls, one max
            for g in range(NG):
                ps = psT.tile([128, 4, 2, 128], DT, tag="psT")
                for m in range(4):
                    b = g * 4 + m
                    for h in range(2):
                        nc.tensor.transpose(ps[:, m, h, :], dn[:, h * BS + b, :], eye_sb)
                dt = dtpool.tile([128, 4, 2, 128], DT, tag="dt")
                nc.scalar.copy(dt.rearrange("p a b c -> p (a b c)"),
                               ps.rearrange("p a b c -> p (a b c)"))

                sim = psMM.tile([128, LD], F32, tag="sim")
                for m in range(4):
                    b = g * 4 + m
                    nc.tensor.matmul(sim[m * 32:(m + 1) * 32, :],
                                     lhsT=qT[:, b * 32:(b + 1) * 32],
                                     rhs=dt.rearrange("p a b c -> p (a b c)")[:, m * 256:(m + 1) * 256],
                                     start=True, stop=True,
                                     tile_position=(0, m * 32))
                nc.vector.reduce_max(out=maxs[:, g * NWAY + n:g * NWAY + n + 1],
                                     in_=sim[:, :], axis=AX.X)

        # ---- scores = per-row sum of maxes, scaled by 1/||q|| ------------
        for g in range(NG):
            nc.vector.tensor_scalar_mul(maxs[:, g * NWAY:(g + 1) * NWAY],
                                        maxs[:, g * NWAY:(g + 1) * NWAY],
                                        invq[:, g:g + 1])
        scores_ps = psS.tile([4, NG * NWAY], F32, tag="scores")
        nc.tensor.matmul(scores_ps, lhsT=blockones, rhs=maxs, start=True, stop=True)
        sc = singles.tile([4, NG * NWAY], F32)  # [m, g*8+n] = scores[b=g*4+m, n]
        nc.vector.tensor_copy(sc, scores_ps)

        # ---- softmax over n (per g-slice) --------------------------------
        rm = singles.tile([4, NG], F32)
        sm = singles.tile([4, NG], F32)
        for g in range(NG):
            gs = slice(g * NWAY, (g + 1) * NWAY)
            nc.vector.reduce_max(out=rm[:, g:g + 1], in_=sc[:, gs], axis=AX.X)
        for g in range(NG):
            gs = slice(g * NWAY, (g + 1) * NWAY)
            nc.vector.tensor_scalar_sub(sc[:, gs], sc[:, gs], rm[:, g:g + 1])
        nc.scalar.activation(out=sc, in_=sc, func=AF.Exp)
        for g in range(NG):
            gs = slice(g * NWAY, (g + 1) * NWAY)
            nc.vector.reduce_sum(out=sm[:, g:g + 1], in_=sc[:, gs], axis=AX.X)
        nc.vector.reciprocal(sm, sm)
        for g in range(NG):
            gs = slice(g * NWAY, (g + 1) * NWAY)
            nc.vector.tensor_scalar_mul(sc[:, gs], sc[:, gs], sm[:, g:g + 1])
        # sc now holds p = softmax(scores)

        # ---- ConvexSH loss ----------------------------------------------
        t3 = lab_sb[:, :, 0:NWAY]
        r3 = lab_sb[:, :, NWAY:2 * NWAY]
        w3 = lab_sb[:, :, 2 * NWAY:3 * NWAY]

        def t32(name):
            t = singles.tile([4, NG * NWAY], F32, tag=name)
            return t, t.rearrange("p (g n) -> p g n", g=NG)

        a, a3 = t32("a")        # 2w - 1
        b1, b13 = t32("b1")     # 1 - w
        nc.vector.tensor_scalar(out=a3, in0=w3, scalar1=2.0, scalar2=-1.0,
                                op0=ALU.mult, op1=ALU.add)
        nc.vector.tensor_scalar(out=b13, in0=w3, scalar1=-1.0, scalar2=1.0,
                                op0=ALU.mult, op1=ALU.add)

        p2, _ = t32("p2")
        nc.vector.tensor_mul(p2, a, sc)
        nc.vector.tensor_add(p2, p2, b1)
        tinv, tinv3 = t32("tinv")
        nc.vector.tensor_mul(tinv3, a3, t3)
        nc.vector.tensor_add(tinv, tinv, b1)

        lp, _ = t32("lp")
        nc.scalar.activation(out=lp, in_=p2, func=AF.Ln)
        losses, losses3 = t32("losses")
        nc.scalar.activation(out=losses, in_=tinv, func=AF.Ln)  # ln(t_inv)
        nc.vector.tensor_sub(losses, losses, lp)                # ln(t_inv) - ln(p2)
        nc.vector.tensor_mul(losses3, losses3, t3)              # * teacher

        rr, rr3 = t32("rr")
        nc.vector.reciprocal(rr3, r3)
        srr0 = singles.tile([4, NG], F32)
        nc.vector.tensor_scalar_mul(srr0, rr.rearrange("p (g n) -> p g n", g=NG)[:, :, 0], ALPHA)
        wts, _ = t32("wts")
        nc.vector.tensor_scalar(out=wts, in0=rr, scalar1=-ALPHA, scalar2=GAMMA,
                                op0=ALU.mult, op1=ALU.add)
        for g in range(NG):
            gs = slice(g * NWAY, (g + 1) * NWAY)
            nc.vector.tensor_scalar_add(wts[:, gs], wts[:, gs], srr0[:, g:g + 1])

        omp2, _ = t32("omp2")   # 1 - p2
        nc.vector.tensor_scalar(out=omp2, in0=p2, scalar1=-1.0, scalar2=1.0,
                                op0=ALU.mult, op1=ALU.add)
        pw1, _ = t32("pw1")     # (1-p2) ** wts
        nc.scalar.activation(out=pw1, in_=omp2, func=AF.Ln)
        nc.vector.tensor_mul(pw1, pw1, wts)
        nc.scalar.activation(out=pw1, in_=pw1, func=AF.Exp)
        pw2, _ = t32("pw2")     # p2 ** wts
        nc.vector.tensor_mul(pw2, lp, wts)
        nc.scalar.activation(out=pw2, in_=pw2, func=AF.Exp)

        lv, lv3 = t32("lv")
        nc.vector.tensor_mul(lv3, w3, pw1.rearrange("p (g n) -> p g n", g=NG))
        t2, t23 = t32("t2")
        nc.vector.tensor_mul(t23, b13, pw2.rearrange("p (g n) -> p g n", g=NG))
        nc.vector.tensor_add(lv, lv, t2)
        nc.vector.tensor_mul(lv, lv, losses)

        partial = singles.tile([4, 1], F32)
        nc.vector.reduce_sum(out=partial, in_=lv, axis=AX.X)
        out_ps = psS.tile([1, 1], F32, tag="out")
        nc.tensor.matmul(out_ps, lhsT=ones4, rhs=partial, start=True, stop=True)
        out_sb = singles.tile([1, 1], F32)
        nc.vector.tensor_copy(out_sb, out_ps)
        nc.sync.dma_start(out=y_d[:, :], in_=out_sb)

    nc.finalize()
    return nc


_nc_cache = None


def kernel(query_reps, doc_reps, doc_masks, labels):
    global _nc_cache, LAST_RESULTS
    if _nc_cache is None:
        _nc_cache = _build()
    nc = _nc_cache

    eye = np.eye(128, dtype=np.float32)
    in_maps = []
    for c in range(NCORES):
        sl = slice(c * BS, (c + 1) * BS)
        in_maps.append({
            "q": np.ascontiguousarray(query_reps[sl]).astype(np.float32, copy=False),
            "doc": np.ascontiguousarray(doc_reps[:, sl]).astype(np.float32, copy=False),
            "mask": np.ascontiguousarray(doc_masks[:, sl]).astype(np.float32, copy=False),
            "lab": np.ascontiguousarray(labels[sl]).astype(np.float32, copy=False),
            "eye": eye,
        })

    kwargs = {}
    if TRACE:
        kwargs["trace"] = True
    res = run_bass_kernel_spmd(nc, in_maps, core_ids=list(range(NCORES)), **kwargs)
    LAST_RESULTS = res
    total = sum(float(res.results[c]["y"][0, 0]) for c in range(NCORES))
    return np.array(total / (B * NWAY), dtype=np.float32)



# revision 6
# speedup vs baseline: 1.3657x; 1.3657x over previous
"""ConvexSH ColBERT loss kernel for 8 trn2 NeuronCores (v3).

Shards batch B=128 over 8 cores (16 rows each); host averages the 8 partial
sums. Doc layout per candidate n: one fully CONTIGUOUS 2 MB SWDGE cast-DMA
(f32->bf16) into [128, 32, 128] where partition p = b*8 + e covers tokens
k = e*32 + k'. Global k order is permuted - harmless under MaxSim's max.

v3: software-pipelined emission so per-engine program order never couples
block i's early stages to block i-1's late stages:
  iteration i emits  square(i+1) | reduce/sqrt/recip/scale/norm(i) |
  transposes+evac+matmul+max(i-1).
ssq reduce outputs bf16 (2x DVE mode, f32 internal accum); reciprocal is the
approx-fast custom op; transposes/evacs go in 4 quarter-tiles with a tunable
ACT/DVE split; label-only loss terms are precomputed on host; tail groups
Ln/Exp uses to bound table swaps.
"""

import sys
from contextlib import ExitStack

import numpy as np

for _p in ("/opt/trn_rl_repo", "/root/.axon_site/_ro/trn_rl_repo"):
    if _p not in sys.path:
        sys.path.append(_p)

import concourse.bacc as bacc
import concourse.tile as tile
from concourse import mybir
from concourse.bass_utils import run_bass_kernel_spmd

AF = mybir.ActivationFunctionType
AX = mybir.AxisListType
ALU = mybir.AluOpType
F32 = mybir.dt.float32
BF16 = mybir.dt.bfloat16

NCORES = 8
B, LQ, LD, D, NWAY = 128, 32, 256, 128, 8
BS = B // NCORES  # 16 batch rows per core
NG = BS // 4      # 4 groups of 4 rows
NE = LD // 32     # 8 eighths of tokens per row -> partition p = b*8 + e
KT = 32           # tokens per partition (k')
ALPHA, GAMMA = 0.2, 2.0

REDUCE_BF16 = True      # ssq reduce writes bf16 (enables DVE 2x path)
EVAC_DVE_QUARTERS = 1   # how many of the 4 evac quarters go to DVE (rest ACT)

TRACE = False
LAST_RESULTS = None


def _build():
    nc = bacc.Bacc("TRN2", target_bir_lowering=False, detect_race_conditions=False)

    q_d = nc.dram_tensor("q", [BS, LQ, D], F32, kind="ExternalInput")
    doc_d = nc.dram_tensor("doc", [NWAY, BS, LD, D], F32, kind="ExternalInput")
    mask_d = nc.dram_tensor("mask", [NWAY, BS, LD], F32, kind="ExternalInput")
    lab_d = nc.dram_tensor("lab", [BS, 6 * NWAY], F32, kind="ExternalInput")
    eye_d = nc.dram_tensor("eye", [128, 128], F32, kind="ExternalInput")
    y_d = nc.dram_tensor("y", [1, 1], F32, kind="ExternalOutput")

    with tile.TileContext(nc) as tc, ExitStack() as ctx:
        singles = ctx.enter_context(tc.tile_pool(name="singles", bufs=1))
        dnp = ctx.enter_context(tc.tile_pool(name="dnp", bufs=4))
        sqp = ctx.enter_context(tc.tile_pool(name="sqp", bufs=2))
        dtp = ctx.enter_context(tc.tile_pool(name="dtp", bufs=2))
        smp = ctx.enter_context(tc.tile_pool(name="smp", bufs=2))
        psT = ctx.enter_context(tc.tile_pool(name="psT", bufs=3, space="PSUM"))
        psS = ctx.enter_context(tc.tile_pool(name="psS", bufs=2, space="PSUM"))

        # ---- constants -----------------------------------------------------
        eye_f = singles.tile([128, 128], F32)
        nc.sync.dma_start(out=eye_f, in_=eye_d[:, :])
        eye_bf = singles.tile([128, 128], BF16)
        nc.vector.tensor_copy(eye_bf, eye_f)

        blockones = singles.tile([128, NG], F32)
        nc.vector.memset(blockones, 0.0)
        for m in range(4):
            nc.vector.memset(blockones[m * 32:(m + 1) * 32, m:m + 1], 1.0)
        ones4 = singles.tile([4, 1], F32)
        nc.vector.memset(ones4, 1.0)

        # host-precomputed label constants: [t, a, b1, lnt, wts, w] x NWAY
        lab_sb = singles.tile([4, NG, 6 * NWAY], F32)
        nc.sync.dma_start(out=lab_sb, in_=lab_d.rearrange("(g m) c -> m g c", m=4))

        # masks in the (b, e) x (n, k') layout, cast to bf16
        mask_f = singles.tile([128, NWAY, KT], F32)
        nc.sync.dma_start(out=mask_f,
                          in_=mask_d.rearrange("n b (e k) -> (b e) n k", e=NE))
        mask_b = singles.tile([128, NWAY, KT], BF16)
        nc.vector.tensor_copy(mask_b, mask_f)

        # ---- query path ----------------------------------------------------
        q_f32 = singles.tile([128, NG, D], F32)
        nc.sync.dma_start(out=q_f32, in_=q_d.rearrange("(g m) q d -> (m q) g d", m=4))
        q_nat = singles.tile([128, NG, D], BF16)
        nc.vector.tensor_copy(q_nat, q_f32)

        ssq_q = singles.tile([128, NG], F32)
        qsq = singles.tile([128, D], BF16)
        for g in range(NG):
            nc.vector.scalar_tensor_tensor(
                out=qsq, in0=q_nat[:, g, :], scalar=1.0, in1=q_nat[:, g, :],
                op0=ALU.mult, op1=ALU.mult,
                accum_out=ssq_q[:, g:g + 1])
        invq = singles.tile([128, NG], F32)
        nc.scalar.activation(out=invq, in_=ssq_q, func=AF.Sqrt)
        nc.vector.reciprocal(invq, invq)

        qT = singles.tile([128, NG, 128], BF16)
        ps_q = psT.tile([128, NG, 128], BF16, tag="psq", bufs=1)
        for g in range(NG):
            nc.tensor.transpose(ps_q[:, g, :], q_nat[:, g, :], eye_bf)
        nc.scalar.copy(qT.rearrange("p a b -> p (a b)"),
                       ps_q.rearrange("p a b -> p (a b)"))

        maxs = singles.tile([128, NG, NWAY], F32)

        # ---- software-pipelined main loop ---------------------------------
        SSQ_DT = BF16 if REDUCE_BF16 else F32
        state = {}

        def stage_dma(n):
            dn = dnp.tile([128, KT, D], BF16, tag="dn", name=f"dn{n}")
            nc.gpsimd.dma_start(
                out=dn.rearrange("p t d -> p (t d)"),
                in_=doc_d[n].rearrange("b (e t) d -> (b e) (t d)", e=NE))
            state[n] = {"dn": dn}

        def stage_square(n):
            sq = sqp.tile([128, KT, D], BF16, tag="sq", name=f"sq{n}")
            nc.scalar.activation(out=sq.rearrange("p t d -> p (t d)"),
                                 in_=state[n]["dn"].rearrange("p t d -> p (t d)"),
                                 func=AF.Square)
            state[n]["sq"] = sq

        def stage_norm(n):
            dn, sq = state[n]["dn"], state[n]["sq"]
            ssq = smp.tile([128, KT], SSQ_DT, tag="ssq", name=f"ssq{n}")
            if REDUCE_BF16:
                with nc.allow_low_precision("ssq bf16 out; DVE accumulates fp32"):
                    nc.vector.reduce_sum(out=ssq, in_=sq, axis=AX.X)
            else:
                nc.vector.reduce_sum(out=ssq, in_=sq, axis=AX.X)
            rt = smp.tile([128, KT], F32, tag="rt", name=f"rt{n}")
            nc.scalar.activation(out=rt, in_=ssq, func=AF.Sqrt)
            nc.vector.reciprocal_approx_fast(rt, rt)
            scale2 = smp.tile([128, KT, 2], BF16, tag="scale2", name=f"s2{n}")
            nc.vector.tensor_mul(scale2[:, :, 0], rt, mask_b[:, n, :])
            nc.vector.tensor_copy(scale2[:, :, 1], scale2[:, :, 0])
            dn4 = dn.rearrange("p t (h w) -> p t h w", w=2)
            nc.vector.tensor_tensor(
                out=dn4, in0=dn4,
                in1=scale2.unsqueeze(2).broadcast_to([128, KT, D // 2, 2]),
                op=ALU.mult)

        def stage_sim(n):
            dn = state[n]["dn"]
            dT = dtp.tile([128, KT, 128], BF16, tag="dT", name=f"dT{n}")
            for qt in range(4):
                ps = psT.tile([128, 8, 128], BF16, tag="psT", name=f"ps{n}_{qt}")
                for j in range(8):
                    nc.tensor.transpose(ps[:, j, :], dn[:, qt * 8 + j, :], eye_bf)
                half = dT[:, qt * 8:(qt + 1) * 8, :]
                if qt < 4 - EVAC_DVE_QUARTERS:
                    nc.scalar.copy(half.rearrange("p t d -> p (t d)"),
                                   ps.rearrange("p t d -> p (t d)"))
                else:
                    nc.vector.tensor_copy(half.rearrange("p t d -> p (t d)"),
                                          ps.rearrange("p t d -> p (t d)"))
            sim = psS.tile([128, NG, 256], F32, tag="sim", name=f"sim{n}")
            for g in range(NG):
                for m in range(4):
                    b = g * 4 + m
                    nc.tensor.matmul(sim[m * 32:(m + 1) * 32, g, :],
                                     lhsT=qT[:, g, m * 32:(m + 1) * 32],
                                     rhs=dT[:, :, NE * b:NE * (b + 1)],
                                     start=True, stop=True,
                                     tile_position=(0, m * 32))
            nc.vector.reduce_max(out=maxs[:, :, n], in_=sim, axis=AX.X)
            del state[n]

        # fill: dma 0,1 and square 0 up front
        stage_dma(0)
        stage_dma(1)
        stage_square(0)
        for i in range(NWAY):
            if i + 2 < NWAY:
                stage_dma(i + 2)
            if i + 1 < NWAY:
                stage_square(i + 1)
            stage_norm(i)
            if i >= 1:
                stage_sim(i - 1)
        stage_sim(NWAY - 1)

        # ---- scores --------------------------------------------------------
        for g in range(NG):
            nc.vector.tensor_scalar_mul(maxs[:, g, :], maxs[:, g, :],
                                        invq[:, g:g + 1])
        scores_ps = psT.tile([4, NG * NWAY], F32, tag="psq", bufs=1)
        nc.tensor.matmul(scores_ps, lhsT=blockones,
                         rhs=maxs.rearrange("p g n -> p (g n)"),
                         start=True, stop=True)
        sc = singles.tile([4, NG * NWAY], F32)  # [m, g*8+n]
        nc.vector.tensor_copy(sc, scores_ps)

        # ---- softmax over n (per g-slice); one Exp -------------------------
        rm = singles.tile([4, NG], F32)
        sm = singles.tile([4, NG], F32)
        sc3 = sc.rearrange("p (g n) -> p g n", g=NG)
        nc.vector.reduce_max(out=rm, in_=sc3, axis=AX.X)
        for g in range(NG):
            gs = slice(g * NWAY, (g + 1) * NWAY)
            nc.vector.tensor_scalar_sub(sc[:, gs], sc[:, gs], rm[:, g:g + 1])
        nc.scalar.activation(out=sc, in_=sc, func=AF.Exp)
        nc.vector.reduce_sum(out=sm, in_=sc3, axis=AX.X)
        nc.vector.reciprocal(sm, sm)
        for g in range(NG):
            gs = slice(g * NWAY, (g + 1) * NWAY)
            nc.vector.tensor_scalar_mul(sc[:, gs], sc[:, gs], sm[:, g:g + 1])

        # ---- ConvexSH loss (label-only terms precomputed on host) ---------
        def fld(i):
            return lab_sb[:, :, i * NWAY:(i + 1) * NWAY]
        t3, a3, b13, lnt3, wts3, w3 = (fld(i) for i in range(6))

        def t32(name):
            t = singles.tile([4, NG * NWAY], F32, tag=name)
            return t, t.rearrange("p (g n) -> p g n", g=NG)

        p2, p23 = t32("p2")
        nc.vector.tensor_mul(p23, a3, sc3)
        nc.vector.tensor_add(p23, p23, b13)
        omp2, _ = t32("omp2")   # 1 - p2
        nc.vector.tensor_scalar(out=omp2, in0=p2, scalar1=-1.0, scalar2=1.0,
                                op0=ALU.mult, op1=ALU.add)
        # Ln group
        lp, lp3 = t32("lp")
        nc.scalar.activation(out=lp, in_=p2, func=AF.Ln)
        nc.scalar.activation(out=omp2, in_=omp2, func=AF.Ln)  # ln(1-p2)
        losses, losses3 = t32("losses")
        nc.vector.tensor_sub(losses3, lnt3, lp3)
        nc.vector.tensor_mul(losses3, losses3, t3)
        # Exp group: pw1 = (1-p2)^wts, pw2 = p2^wts
        nc.vector.tensor_mul(omp2.rearrange("p (g n) -> p g n", g=NG),
                             omp2.rearrange("p (g n) -> p g n", g=NG), wts3)
        nc.vector.tensor_mul(lp3, lp3, wts3)
        nc.scalar.activation(out=omp2, in_=omp2, func=AF.Exp)  # pw1
        nc.scalar.activation(out=lp, in_=lp, func=AF.Exp)      # pw2
        lv, lv3 = t32("lv")
        nc.vector.tensor_mul(lv3, w3, omp2.rearrange("p (g n) -> p g n", g=NG))
        t2, t23 = t32("t2")
        nc.vector.tensor_mul(t23, b13, lp3)
        nc.vector.tensor_add(lv, lv, t2)
        nc.vector.tensor_mul(lv, lv, losses)

        partial = singles.tile([4, 1], F32)
        nc.vector.reduce_sum(out=partial, in_=lv, axis=AX.X)
        out_ps = psT.tile([1, 1], F32, tag="psq", bufs=1)
        nc.tensor.matmul(out_ps, lhsT=ones4, rhs=partial, start=True, stop=True)
        out_sb = singles.tile([1, 1], F32)
        nc.vector.tensor_copy(out_sb, out_ps)
        nc.sync.dma_start(out=y_d[:, :], in_=out_sb)

    nc.finalize()
    return nc


_nc_cache = None


def _lab2(labels):
    t = labels[:, :NWAY].astype(np.float64)
    r = labels[:, NWAY:2 * NWAY].astype(np.float64)
    w = labels[:, 2 * NWAY:].astype(np.float64)
    a = 2.0 * w - 1.0
    b1 = 1.0 - w
    tinv = t * w + (1.0 - t) * (1.0 - w)
    lnt = np.log(tinv)
    rr = 1.0 / r
    wts = GAMMA - ALPHA * (rr - rr[:, :1])
    out = np.concatenate([t, a, b1, lnt, wts, w], axis=1)
    return np.ascontiguousarray(out, dtype=np.float32)


def kernel(query_reps, doc_reps, doc_masks, labels):
    global _nc_cache, LAST_RESULTS
    if _nc_cache is None:
        _nc_cache = _build()
    nc = _nc_cache

    eye = np.eye(128, dtype=np.float32)
    labels = np.asarray(labels)
    in_maps = []
    for c in range(NCORES):
        sl = slice(c * BS, (c + 1) * BS)
        in_maps.append({
            "q": np.ascontiguousarray(query_reps[sl]).astype(np.float32, copy=False),
            "doc": np.ascontiguousarray(doc_reps[:, sl]).astype(np.float32, copy=False),
            "mask": np.ascontiguousarray(doc_masks[:, sl]).astype(np.float32, copy=False),
            "lab": _lab2(labels[sl]),
            "eye": eye,
        })

    kwargs = {}
    if TRACE:
        kwargs["trace"] = True
    res = run_bass_kernel_spmd(nc, in_maps, core_ids=list(range(NCORES)), **kwargs)
    LAST_RESULTS = res
    total = sum(float(res.results[c]["y"][0, 0]) for c in range(NCORES))
    return np.array(total / (B * NWAY), dtype=np.float32)


# revision 8
# speedup vs baseline: 1.3839x; 1.0134x over previous
"""ConvexSH ColBERT loss kernel for 8 trn2 NeuronCores (v4).

Shards batch B=128 over 8 cores (16 rows each); host averages the 8 partial
sums. Doc layout per candidate n: one fully CONTIGUOUS 2 MB SWDGE cast-DMA
(f32->bf16) into [128, 32, 128] where partition p = b*8 + e covers tokens
k = e*32 + k'. Global k order is permuted - harmless under MaxSim's max.

Software-pipelined emission: iteration i emits square(i+1) | norm-chain(i) |
transposes+evac+matmul+max(i-1), so no engine's program order couples a
block's early stages to the previous block's late stages.

ssq = ACT Square -> two DVE pair-adds at 2x (TENSOR_REDUCE has no 2x mode,
tensor_tensor does) -> small 1x reduce. Normalize is one DVE tensor_tensor
with a pair-broadcast scale AP. Transposes/evacs go in 4 quarter-tiles with
a tunable ACT/DVE split. Label-only loss terms are precomputed on host; the
tail runs on flat [4,32] views with broadcast APs, and a manually emitted
InstLoadActFuncSet(natural_log_exp_and_others) after the last Sqrt hides
the tail's activation-table swap behind the final blocks.
"""

import sys
from contextlib import ExitStack

import numpy as np

for _p in ("/opt/trn_rl_repo", "/root/.axon_site/_ro/trn_rl_repo"):
    if _p not in sys.path:
        sys.path.append(_p)

import concourse.bacc as bacc
import concourse.tile as tile
from concourse import mybir
from concourse.bass_utils import run_bass_kernel_spmd

AF = mybir.ActivationFunctionType
AX = mybir.AxisListType
ALU = mybir.AluOpType
F32 = mybir.dt.float32
BF16 = mybir.dt.bfloat16

NCORES = 8
B, LQ, LD, D, NWAY = 128, 32, 256, 128, 8
BS = B // NCORES  # 16 batch rows per core
NG = BS // 4      # 4 groups of 4 rows
NE = LD // 32     # 8 eighths of tokens per row -> partition p = b*8 + e
KT = 32           # tokens per partition (k')
ALPHA, GAMMA = 0.2, 2.0

EVAC_DVE_QUARTERS = 1   # how many of the 4 evac quarters go to DVE (rest ACT)
NLE_SET_ID = 6          # natural_log_exp_and_others in act_info.json

TRACE = False
LAST_RESULTS = None


def _build():
    nc = bacc.Bacc("TRN2", target_bir_lowering=False, detect_race_conditions=False)

    q_d = nc.dram_tensor("q", [BS, LQ, D], F32, kind="ExternalInput")
    doc_d = nc.dram_tensor("doc", [NWAY, BS, LD, D], F32, kind="ExternalInput")
    mask_d = nc.dram_tensor("mask", [NWAY, BS, LD], F32, kind="ExternalInput")
    lab_d = nc.dram_tensor("lab", [BS, 6 * NWAY], F32, kind="ExternalInput")
    eye_d = nc.dram_tensor("eye", [128, 128], F32, kind="ExternalInput")
    y_d = nc.dram_tensor("y", [1, 1], F32, kind="ExternalOutput")

    with tile.TileContext(nc) as tc, ExitStack() as ctx:
        singles = ctx.enter_context(tc.tile_pool(name="singles", bufs=1))
        dnp = ctx.enter_context(tc.tile_pool(name="dnp", bufs=6))
        sqp = ctx.enter_context(tc.tile_pool(name="sqp", bufs=2))
        dtp = ctx.enter_context(tc.tile_pool(name="dtp", bufs=2))
        smp = ctx.enter_context(tc.tile_pool(name="smp", bufs=2))
        psT = ctx.enter_context(tc.tile_pool(name="psT", bufs=4, space="PSUM"))
        psS = ctx.enter_context(tc.tile_pool(name="psS", bufs=2, space="PSUM"))

        # ---- constants -----------------------------------------------------
        eye_f = singles.tile([128, 128], F32)
        nc.sync.dma_start(out=eye_f, in_=eye_d[:, :])
        eye_bf = singles.tile([128, 128], BF16)
        nc.vector.tensor_copy(eye_bf, eye_f)

        blockones = singles.tile([128, NG], F32)
        nc.vector.memset(blockones, 0.0)
        for m in range(4):
            nc.vector.memset(blockones[m * 32:(m + 1) * 32, m:m + 1], 1.0)
        ones4 = singles.tile([4, 1], F32)
        nc.vector.memset(ones4, 1.0)

        # host-precomputed label constants: [t, a, b1, lnt, wts, w] x NWAY
        lab_sb = singles.tile([4, NG, 6 * NWAY], F32)
        nc.sync.dma_start(out=lab_sb, in_=lab_d.rearrange("(g m) c -> m g c", m=4))

        # masks in the (b, e) x (n, k') layout, cast to bf16
        mask_f = singles.tile([128, NWAY, KT], F32)
        nc.sync.dma_start(out=mask_f,
                          in_=mask_d.rearrange("n b (e k) -> (b e) n k", e=NE))
        mask_b = singles.tile([128, NWAY, KT], BF16)
        nc.vector.tensor_copy(mask_b, mask_f)

        # ---- query path ----------------------------------------------------
        q_f32 = singles.tile([128, NG, D], F32)
        nc.sync.dma_start(out=q_f32, in_=q_d.rearrange("(g m) q d -> (m q) g d", m=4))
        q_nat = singles.tile([128, NG, D], BF16)
        nc.vector.tensor_copy(q_nat, q_f32)

        ssq_q = singles.tile([128, NG], F32)
        qsq = singles.tile([128, D], BF16)
        for g in range(NG):
            nc.vector.scalar_tensor_tensor(
                out=qsq, in0=q_nat[:, g, :], scalar=1.0, in1=q_nat[:, g, :],
                op0=ALU.mult, op1=ALU.mult,
                accum_out=ssq_q[:, g:g + 1])
        invq = singles.tile([128, NG], F32)
        nc.scalar.activation(out=invq, in_=ssq_q, func=AF.Sqrt)
        nc.vector.reciprocal(invq, invq)

        qT = singles.tile([128, NG, 128], BF16)
        ps_q = psT.tile([128, 8, 128], BF16, tag="psT")
        for g in range(NG):
            nc.tensor.transpose(ps_q[:, g, :], q_nat[:, g, :], eye_bf)
        nc.scalar.copy(qT.rearrange("p a b -> p (a b)"),
                       ps_q[:, 0:NG, :].rearrange("p a b -> p (a b)"))

        maxs = singles.tile([128, NG, NWAY], F32)

        # ---- software-pipelined main loop ---------------------------------
        state = {}

        def stage_dma(n):
            dn = dnp.tile([128, KT, D], BF16, tag="dn", name=f"dn{n}")
            nc.gpsimd.dma_start(
                out=dn.rearrange("p t d -> p (t d)"),
                in_=doc_d[n].rearrange("b (e t) d -> (b e) (t d)", e=NE))
            state[n] = {"dn": dn}

        def stage_square(n):
            sq = sqp.tile([128, KT, D], BF16, tag="sq", name=f"sq{n}")
            nc.scalar.activation(out=sq.rearrange("p t d -> p (t d)"),
                                 in_=state[n]["dn"].rearrange("p t d -> p (t d)"),
                                 func=AF.Square)
            state[n]["sq"] = sq

        def stage_norm(n):
            dn, sq = state[n]["dn"], state[n]["sq"]
            # two pair-add stages at DVE 2x, then a small 1x reduce
            nc.vector.tensor_add(sq[:, :, 0:64], sq[:, :, 0:64], sq[:, :, 64:128])
            nc.vector.tensor_add(sq[:, :, 0:32], sq[:, :, 0:32], sq[:, :, 32:64])
            ssq = smp.tile([128, KT], F32, tag="ssq", name=f"ssq{n}")
            nc.vector.reduce_sum(out=ssq, in_=sq[:, :, 0:32], axis=AX.X)
            rt = smp.tile([128, KT], F32, tag="rt", name=f"rt{n}")
            nc.vector.reciprocal_approx_fast(rt, ssq)
            nc.scalar.activation(out=rt, in_=rt, func=AF.Sqrt)  # 1/||d||
            scale2 = smp.tile([128, KT, 2], BF16, tag="scale2", name=f"s2{n}")
            nc.vector.tensor_mul(scale2[:, :, 0], rt, mask_b[:, n, :])
            nc.vector.tensor_copy(scale2[:, :, 1], scale2[:, :, 0])
            dn4 = dn.rearrange("p t (h w) -> p t h w", w=2)
            nc.vector.tensor_tensor(
                out=dn4, in0=dn4,
                in1=scale2.unsqueeze(2).broadcast_to([128, KT, D // 2, 2]),
                op=ALU.mult)

        def stage_sim(n):
            dn = state[n]["dn"]
            dT = dtp.tile([128, KT, 128], BF16, tag="dT", name=f"dT{n}")
            for qt in range(4):
                ps = psT.tile([128, 8, 128], BF16, tag="psT", name=f"ps{n}_{qt}")
                for j in range(8):
                    nc.tensor.transpose(ps[:, j, :], dn[:, qt * 8 + j, :], eye_bf)
                quarter = dT[:, qt * 8:(qt + 1) * 8, :]
                if qt < 4 - EVAC_DVE_QUARTERS:
                    nc.scalar.copy(quarter.rearrange("p t d -> p (t d)"),
                                   ps.rearrange("p t d -> p (t d)"))
                else:
                    nc.vector.tensor_copy(quarter.rearrange("p t d -> p (t d)"),
                                          ps.rearrange("p t d -> p (t d)"))
            sim = psS.tile([128, NG, 256], F32, tag="sim", name=f"sim{n}")
            for g in range(NG):
                for m in range(4):
                    b = g * 4 + m
                    nc.tensor.matmul(sim[m * 32:(m + 1) * 32, g, :],
                                     lhsT=qT[:, g, m * 32:(m + 1) * 32],
                                     rhs=dT[:, :, NE * b:NE * (b + 1)],
                                     start=True, stop=True,
                                     tile_position=(0, m * 32))
            nc.vector.reduce_max(out=maxs[:, :, n], in_=sim, axis=AX.X)
            del state[n]

        stage_dma(0)
        stage_dma(1)
        stage_square(0)
        for i in range(NWAY):
            if i + 2 < NWAY:
                stage_dma(i + 2)
            if i + 1 < NWAY:
                stage_square(i + 1)
            stage_norm(i)
            if i == NWAY - 1:
                # preload the tail's exp+ln table; executes right after the
                # last Sqrt, hidden behind the final blocks' transposes
                nc.scalar.add_instruction(mybir.InstLoadActFuncSet(
                    name=nc.get_next_instruction_name(), ins=[], outs=[],
                    act_func_set_id=NLE_SET_ID))
            if i >= 1:
                stage_sim(i - 1)
        stage_sim(NWAY - 1)

        # ---- scores --------------------------------------------------------
        nc.vector.tensor_tensor(
            out=maxs, in0=maxs,
            in1=invq.unsqueeze(2).broadcast_to([128, NG, NWAY]), op=ALU.mult)
        scores_ps = psT.tile([4, NG * NWAY], F32, tag="psT")
        nc.tensor.matmul(scores_ps, lhsT=blockones,
                         rhs=maxs.rearrange("p g n -> p (g n)"),
                         start=True, stop=True)
        sc = singles.tile([4, NG * NWAY], F32)  # [m, g*8+n]
        nc.vector.tensor_copy(sc, scores_ps)

        # ---- softmax over n (per g-slice); one Exp -------------------------
        rm = singles.tile([4, NG], F32)
        sm = singles.tile([4, NG], F32)
        sc3 = sc.rearrange("p (g n) -> p g n", g=NG)
        nc.vector.reduce_max(out=rm, in_=sc3, axis=AX.X)
        nc.vector.tensor_tensor(out=sc3, in0=sc3,
                                in1=rm.unsqueeze(2).broadcast_to([4, NG, NWAY]),
                                op=ALU.subtract)
        nc.scalar.activation(out=sc, in_=sc, func=AF.Exp)
        nc.vector.reduce_sum(out=sm, in_=sc3, axis=AX.X)
        nc.vector.reciprocal(sm, sm)
        nc.vector.tensor_tensor(out=sc3, in0=sc3,
                                in1=sm.unsqueeze(2).broadcast_to([4, NG, NWAY]),
                                op=ALU.mult)

        # ---- ConvexSH loss (label-only terms precomputed on host) ---------
        F = NG * NWAY

        def fld(i):
            return lab_sb[:, :, i * NWAY:(i + 1) * NWAY]
        t3, a3, b13, lnt3, wts3, w3 = (fld(i) for i in range(6))

        def t32(name):
            t = singles.tile([4, F], F32, tag=name)
            return t, t.rearrange("p (g n) -> p g n", g=NG)

        p2, p23 = t32("p2")
        nc.vector.tensor_mul(p23, a3, sc3)
        nc.vector.tensor_add(p23, p23, b13)
        omp2, omp23 = t32("omp2")   # 1 - p2
        nc.vector.tensor_scalar(out=omp2, in0=p2, scalar1=-1.0, scalar2=1.0,
                                op0=ALU.mult, op1=ALU.add)
        lp, lp3 = t32("lp")
        nc.scalar.activation(out=lp, in_=p2, func=AF.Ln)
        nc.scalar.activation(out=omp2, in_=omp2, func=AF.Ln)  # ln(1-p2)
        losses, losses3 = t32("losses")
        nc.vector.tensor_sub(losses3, lnt3, lp3)
        nc.vector.tensor_mul(losses3, losses3, t3)
        nc.vector.tensor_mul(omp23, omp23, wts3)
        nc.vector.tensor_mul(lp3, lp3, wts3)
        nc.scalar.activation(out=omp2, in_=omp2, func=AF.Exp)  # (1-p2)^wts
        nc.scalar.activation(out=lp, in_=lp, func=AF.Exp)      # p2^wts
        lv, lv3 = t32("lv")
        nc.vector.tensor_mul(lv3, w3, omp23)
        t2, t23 = t32("t2")
        nc.vector.tensor_mul(t23, b13, lp3)
        nc.vector.tensor_add(lv, lv, t2)
        nc.vector.tensor_mul(lv, lv, losses)

        partial = singles.tile([4, 1], F32)
        nc.vector.reduce_sum(out=partial, in_=lv, axis=AX.X)
        out_ps = psT.tile([1, 1], F32, tag="psT")
        nc.tensor.matmul(out_ps, lhsT=ones4, rhs=partial, start=True, stop=True)
        out_sb = singles.tile([1, 1], F32)
        nc.vector.tensor_copy(out_sb, out_ps)
        nc.sync.dma_start(out=y_d[:, :], in_=out_sb)

    nc.finalize()
    return nc


_nc_cache = None


def _lab2(labels):
    t = labels[:, :NWAY].astype(np.float64)
    r = labels[:, NWAY:2 * NWAY].astype(np.float64)
    w = labels[:, 2 * NWAY:].astype(np.float64)
    a = 2.0 * w - 1.0
    b1 = 1.0 - w
    tinv = t * w + (1.0 - t) * (1.0 - w)
    lnt = np.log(tinv)
    rr = 1.0 / r
    wts = GAMMA - ALPHA * (rr - rr[:, :1])
    out = np.concatenate([t, a, b1, lnt, wts, w], axis=1)
    return np.ascontiguousarray(out, dtype=np.float32)


def kernel(query_reps, doc_reps, doc_masks, labels):
    global _nc_cache, LAST_RESULTS
    if _nc_cache is None:
        _nc_cache = _build()
    nc = _nc_cache

    eye = np.eye(128, dtype=np.float32)
    labels = np.asarray(labels)
    in_maps = []
    for c in range(NCORES):
        sl = slice(c * BS, (c + 1) * BS)
        in_maps.append({
            "q": np.ascontiguousarray(query_reps[sl]).astype(np.float32, copy=False),
            "doc": np.ascontiguousarray(doc_reps[:, sl]).astype(np.float32, copy=False),
            "mask": np.ascontiguousarray(doc_masks[:, sl]).astype(np.float32, copy=False),
            "lab": _lab2(labels[sl]),
            "eye": eye,
        })

    kwargs = {}
    if TRACE:
        kwargs["trace"] = True
    res = run_bass_kernel_spmd(nc, in_maps, core_ids=list(range(NCORES)), **kwargs)
    LAST_RESULTS = res
    total = sum(float(res.results[c]["y"][0, 0]) for c in range(NCORES))
    return np.array(total / (B * NWAY), dtype=np.float32)


# revision 9
# speedup vs baseline: 1.6832x; 1.2163x over previous
"""ConvexSH ColBERT loss kernel for 8 trn2 NeuronCores (v4).

Shards batch B=128 over 8 cores (16 rows each); host averages the 8 partial
sums. Doc layout per candidate n: one fully CONTIGUOUS 2 MB SWDGE cast-DMA
(f32->bf16) into [128, 32, 128] where partition p = b*8 + e covers tokens
k = e*32 + k'. Global k order is permuted - harmless under MaxSim's max.

Software-pipelined emission: iteration i emits square(i+1) | norm-chain(i) |
transposes+evac+matmul+max(i-1), so no engine's program order couples a
block's early stages to the previous block's late stages.

ssq = ACT Square -> two DVE pair-adds at 2x (TENSOR_REDUCE has no 2x mode,
tensor_tensor does) -> small 1x reduce. Normalize is one DVE tensor_tensor
with a pair-broadcast scale AP. Transposes/evacs go in 4 quarter-tiles with
a tunable ACT/DVE split. Label-only loss terms are precomputed on host; the
tail runs on flat [4,32] views with broadcast APs, and a manually emitted
InstLoadActFuncSet(natural_log_exp_and_others) after the last Sqrt hides
the tail's activation-table swap behind the final blocks.
"""

import sys
from contextlib import ExitStack

import numpy as np

for _p in ("/opt/trn_rl_repo", "/root/.axon_site/_ro/trn_rl_repo"):
    if _p not in sys.path:
        sys.path.append(_p)

import concourse.bacc as bacc
import concourse.tile as tile
from concourse import mybir
from concourse.bass_utils import run_bass_kernel_spmd

AF = mybir.ActivationFunctionType
AX = mybir.AxisListType
ALU = mybir.AluOpType
F32 = mybir.dt.float32
BF16 = mybir.dt.bfloat16

NCORES = 8
B, LQ, LD, D, NWAY = 128, 32, 256, 128, 8
BS = B // NCORES  # 16 batch rows per core
NG = BS // 4      # 4 groups of 4 rows
NE = LD // 32     # 8 eighths of tokens per row -> partition p = b*8 + e
KT = 32           # tokens per partition (k')
ALPHA, GAMMA = 0.2, 2.0

EVAC_DVE_QUARTERS = 1   # how many of the 4 evac quarters go to DVE (rest ACT)
NLE_SET_ID = 6          # natural_log_exp_and_others in act_info.json

TRACE = False
LAST_RESULTS = None


def _build():
    nc = bacc.Bacc("TRN2", target_bir_lowering=False, detect_race_conditions=False)

    q_d = nc.dram_tensor("q", [128, NG, D], F32, kind="ExternalInput")
    doc_d = nc.dram_tensor("doc", [NWAY, BS, LD, D], F32, kind="ExternalInput")
    mask_d = nc.dram_tensor("mask", [128, NWAY, KT], F32, kind="ExternalInput")
    lab_d = nc.dram_tensor("lab", [BS, 6 * NWAY], F32, kind="ExternalInput")
    eye_d = nc.dram_tensor("eye", [128, 128], F32, kind="ExternalInput")
    y_d = nc.dram_tensor("y", [1, 1], F32, kind="ExternalOutput")

    with tile.TileContext(nc) as tc, ExitStack() as ctx:
        singles = ctx.enter_context(tc.tile_pool(name="singles", bufs=1))
        dnp = ctx.enter_context(tc.tile_pool(name="dnp", bufs=6))
        sqp = ctx.enter_context(tc.tile_pool(name="sqp", bufs=2))
        dtp = ctx.enter_context(tc.tile_pool(name="dtp", bufs=2))
        smp = ctx.enter_context(tc.tile_pool(name="smp", bufs=2))
        psT = ctx.enter_context(tc.tile_pool(name="psT", bufs=4, space="PSUM"))
        psS = ctx.enter_context(tc.tile_pool(name="psS", bufs=2, space="PSUM"))

        # ---- constants -----------------------------------------------------
        eye_f = singles.tile([128, 128], F32)
        nc.sync.dma_start(out=eye_f, in_=eye_d[:, :])
        eye_bf = singles.tile([128, 128], BF16)
        nc.vector.tensor_copy(eye_bf, eye_f)

        blockones = singles.tile([128, NG], F32)
        nc.vector.memset(blockones, 0.0)
        for m in range(4):
            nc.vector.memset(blockones[m * 32:(m + 1) * 32, m:m + 1], 1.0)
        ones4 = singles.tile([4, 1], F32)
        nc.vector.memset(ones4, 1.0)

        # host-precomputed label constants: [t, a, b1, lnt, wts, w] x NWAY
        lab_sb = singles.tile([4, NG, 6 * NWAY], F32)
        nc.sync.dma_start(out=lab_sb, in_=lab_d.rearrange("(g m) c -> m g c", m=4))

        # masks in the (b, e) x (n, k') layout, cast to bf16
        mask_f = singles.tile([128, NWAY, KT], F32)
        nc.sync.dma_start(out=mask_f, in_=mask_d[:, :, :])
        mask_b = singles.tile([128, NWAY, KT], BF16)
        nc.vector.tensor_copy(mask_b, mask_f)

        # ---- query path ----------------------------------------------------
        q_f32 = singles.tile([128, NG, D], F32)
        nc.sync.dma_start(out=q_f32, in_=q_d[:, :, :])
        q_nat = singles.tile([128, NG, D], BF16)
        nc.vector.tensor_copy(q_nat, q_f32)

        ssq_q = singles.tile([128, NG], F32)
        qsq = singles.tile([128, D], BF16)
        for g in range(NG):
            nc.vector.scalar_tensor_tensor(
                out=qsq, in0=q_nat[:, g, :], scalar=1.0, in1=q_nat[:, g, :],
                op0=ALU.mult, op1=ALU.mult,
                accum_out=ssq_q[:, g:g + 1])
        invq = singles.tile([128, NG], F32)
        nc.scalar.activation(out=invq, in_=ssq_q, func=AF.Sqrt)
        nc.vector.reciprocal(invq, invq)

        qT = singles.tile([128, NG, 128], BF16)
        ps_q = psT.tile([128, 8, 128], BF16, tag="psT")
        for g in range(NG):
            nc.tensor.transpose(ps_q[:, g, :], q_nat[:, g, :], eye_bf)
        nc.scalar.copy(qT.rearrange("p a b -> p (a b)"),
                       ps_q[:, 0:NG, :].rearrange("p a b -> p (a b)"))

        maxs = singles.tile([128, NG, NWAY], F32)

        # ---- software-pipelined main loop ---------------------------------
        state = {}

        def stage_dma(n):
            dn = dnp.tile([128, KT, D], BF16, tag="dn", name=f"dn{n}")
            nc.gpsimd.dma_start(
                out=dn.rearrange("p t d -> p (t d)"),
                in_=doc_d[n].rearrange("b (e t) d -> (b e) (t d)", e=NE))
            state[n] = {"dn": dn}

        def stage_square(n):
            sq = sqp.tile([128, KT, D], BF16, tag="sq", name=f"sq{n}")
            nc.scalar.activation(out=sq.rearrange("p t d -> p (t d)"),
                                 in_=state[n]["dn"].rearrange("p t d -> p (t d)"),
                                 func=AF.Square)
            state[n]["sq"] = sq

        def stage_norm(n):
            dn, sq = state[n]["dn"], state[n]["sq"]
            # two pair-add stages at DVE 2x, then a small 1x reduce
            nc.vector.tensor_add(sq[:, :, 0:64], sq[:, :, 0:64], sq[:, :, 64:128])
            nc.vector.tensor_add(sq[:, :, 0:32], sq[:, :, 0:32], sq[:, :, 32:64])
            nc.vector.tensor_add(sq[:, :, 0:16], sq[:, :, 0:16], sq[:, :, 16:32])
            ssq = smp.tile([128, KT], F32, tag="ssq", name=f"ssq{n}")
            nc.vector.reduce_sum(out=ssq, in_=sq[:, :, 0:16], axis=AX.X)
            rt = smp.tile([128, KT], F32, tag="rt", name=f"rt{n}")
            nc.vector.reciprocal_approx_fast(rt, ssq)
            nc.scalar.activation(out=rt, in_=rt, func=AF.Sqrt)  # 1/||d||
            scale2 = smp.tile([128, KT, 2], BF16, tag="scale2", name=f"s2{n}")
            nc.vector.tensor_mul(scale2[:, :, 0], rt, mask_b[:, n, :])
            nc.scalar.copy(scale2[:, :, 1], scale2[:, :, 0])
            dn4 = dn.rearrange("p t (h w) -> p t h w", w=2)
            nc.vector.tensor_tensor(
                out=dn4, in0=dn4,
                in1=scale2.unsqueeze(2).broadcast_to([128, KT, D // 2, 2]),
                op=ALU.mult)

        def stage_sim(n):
            dn = state[n]["dn"]
            dT = dtp.tile([128, KT, 128], BF16, tag="dT", name=f"dT{n}")
            for qt in range(4):
                ps = psT.tile([128, 8, 128], BF16, tag="psT", name=f"ps{n}_{qt}")
                for j in range(8):
                    nc.tensor.transpose(ps[:, j, :], dn[:, qt * 8 + j, :], eye_bf)
                quarter = dT[:, qt * 8:(qt + 1) * 8, :]
                if qt < 4 - EVAC_DVE_QUARTERS:
                    nc.scalar.copy(quarter.rearrange("p t d -> p (t d)"),
                                   ps.rearrange("p t d -> p (t d)"))
                else:
                    nc.vector.tensor_copy(quarter.rearrange("p t d -> p (t d)"),
                                          ps.rearrange("p t d -> p (t d)"))
            sim = psS.tile([128, NG, 256], F32, tag="sim", name=f"sim{n}")
            for g in range(NG):
                for m in range(4):
                    b = g * 4 + m
                    nc.tensor.matmul(sim[m * 32:(m + 1) * 32, g, :],
                                     lhsT=qT[:, g, m * 32:(m + 1) * 32],
                                     rhs=dT[:, :, NE * b:NE * (b + 1)],
                                     start=True, stop=True,
                                     tile_position=(0, m * 32))
            nc.vector.reduce_max(out=maxs[:, :, n], in_=sim, axis=AX.X)
            del state[n]

        stage_dma(0)
        stage_dma(1)
        stage_square(0)
        for i in range(NWAY):
            if i + 2 < NWAY:
                stage_dma(i + 2)
            if i + 1 < NWAY:
                stage_square(i + 1)
            stage_norm(i)
            if i >= 1:
                stage_sim(i - 1)
        stage_sim(NWAY - 1)

        # ---- scores --------------------------------------------------------
        nc.vector.tensor_tensor(
            out=maxs, in0=maxs,
            in1=invq.unsqueeze(2).broadcast_to([128, NG, NWAY]), op=ALU.mult)
        scores_ps = psT.tile([4, NG * NWAY], F32, tag="psT")
        nc.tensor.matmul(scores_ps, lhsT=blockones,
                         rhs=maxs.rearrange("p g n -> p (g n)"),
                         start=True, stop=True)
        sc = singles.tile([4, NG * NWAY], F32)  # [m, g*8+n]
        nc.vector.tensor_copy(sc, scores_ps)

        # ---- softmax over n (per g-slice); one Exp -------------------------
        rm = singles.tile([4, NG], F32)
        sm = singles.tile([4, NG], F32)
        sc3 = sc.rearrange("p (g n) -> p g n", g=NG)
        nc.vector.reduce_max(out=rm, in_=sc3, axis=AX.X)
        nc.vector.tensor_tensor(out=sc3, in0=sc3,
                                in1=rm.unsqueeze(2).broadcast_to([4, NG, NWAY]),
                                op=ALU.subtract)
        nc.scalar.activation(out=sc, in_=sc, func=AF.Exp)
        nc.vector.reduce_sum(out=sm, in_=sc3, axis=AX.X)
        nc.vector.reciprocal(sm, sm)
        nc.vector.tensor_tensor(out=sc3, in0=sc3,
                                in1=sm.unsqueeze(2).broadcast_to([4, NG, NWAY]),
                                op=ALU.mult)

        # ---- ConvexSH loss (label-only terms precomputed on host) ---------
        F = NG * NWAY

        def fld(i):
            return lab_sb[:, :, i * NWAY:(i + 1) * NWAY]
        t3, a3, b13, lnt3, wts3, w3 = (fld(i) for i in range(6))

        def t32(name):
            t = singles.tile([4, F], F32, tag=name)
            return t, t.rearrange("p (g n) -> p g n", g=NG)

        p2, p23 = t32("p2")
        nc.vector.tensor_mul(p23, a3, sc3)
        nc.vector.tensor_add(p23, p23, b13)
        omp2, omp23 = t32("omp2")   # 1 - p2
        nc.vector.tensor_scalar(out=omp2, in0=p2, scalar1=-1.0, scalar2=1.0,
                                op0=ALU.mult, op1=ALU.add)
        lp, lp3 = t32("lp")
        nc.scalar.activation(out=lp, in_=p2, func=AF.Ln)
        nc.scalar.activation(out=omp2, in_=omp2, func=AF.Ln)  # ln(1-p2)
        losses, losses3 = t32("losses")
        nc.vector.tensor_sub(losses3, lnt3, lp3)
        nc.vector.tensor_mul(losses3, losses3, t3)
        nc.vector.tensor_mul(omp23, omp23, wts3)
        nc.vector.tensor_mul(lp3, lp3, wts3)
        nc.scalar.activation(out=omp2, in_=omp2, func=AF.Exp)  # (1-p2)^wts
        nc.scalar.activation(out=lp, in_=lp, func=AF.Exp)      # p2^wts
        lv, lv3 = t32("lv")
        nc.vector.tensor_mul(lv3, w3, omp23)
        t2, t23 = t32("t2")
        nc.vector.tensor_mul(t23, b13, lp3)
        nc.vector.tensor_add(lv, lv, t2)
        nc.vector.tensor_mul(lv, lv, losses)

        partial = singles.tile([4, 1], F32)
        nc.vector.reduce_sum(out=partial, in_=lv, axis=AX.X)
        out_ps = psT.tile([1, 1], F32, tag="psT")
        nc.tensor.matmul(out_ps, lhsT=ones4, rhs=partial, start=True, stop=True)
        out_sb = singles.tile([1, 1], F32)
        nc.vector.tensor_copy(out_sb, out_ps)
        nc.sync.dma_start(out=y_d[:, :], in_=out_sb)

    nc.finalize()
    return nc


_nc_cache = None


def _q2(q):
    # [(m q), g, d] so the device upload is one contiguous 2 KB/partition DMA
    return np.ascontiguousarray(
        q.reshape(NG, 4, LQ, D).transpose(1, 2, 0, 3).reshape(128, NG, D))


def _m2(m):
    # [(b e), n, k'] matching the contiguous doc layout
    return np.ascontiguousarray(
        m.reshape(NWAY, BS, NE, KT).transpose(1, 2, 0, 3).reshape(128, NWAY, KT))


def _lab2(labels):
    t = labels[:, :NWAY].astype(np.float64)
    r = labels[:, NWAY:2 * NWAY].astype(np.float64)
    w = labels[:, 2 * NWAY:].astype(np.float64)
    a = 2.0 * w - 1.0
    b1 = 1.0 - w
    tinv = t * w + (1.0 - t) * (1.0 - w)
    lnt = np.log(tinv)
    rr = 1.0 / r
    wts = GAMMA - ALPHA * (rr - rr[:, :1])
    out = np.concatenate([t, a, b1, lnt, wts, w], axis=1)
    return np.ascontiguousarray(out, dtype=np.float32)


def kernel(query_reps, doc_reps, doc_masks, labels):
    global _nc_cache, LAST_RESULTS
    if _nc_cache is None:
        _nc_cache = _build()
    nc = _nc_cache

    eye = np.eye(128, dtype=np.float32)
    labels = np.asarray(labels)
    in_maps = []
    for c in range(NCORES):
        sl = slice(c * BS, (c + 1) * BS)
        in_maps.append({
            "q": _q2(np.asarray(query_reps[sl], dtype=np.float32)),
            "doc": np.ascontiguousarray(doc_reps[:, sl]).astype(np.float32, copy=False),
            "mask": _m2(np.asarray(doc_masks[:, sl], dtype=np.float32)),
            "lab": _lab2(labels[sl]),
            "eye": eye,
        })

    kwargs = {}
    if TRACE:
        kwargs["trace"] = True
    res = run_bass_kernel_spmd(nc, in_maps, core_ids=list(range(NCORES)), **kwargs)
    LAST_RESULTS = res
    total = sum(float(res.results[c]["y"][0, 0]) for c in range(NCORES))
    return np.array(total / (B * NWAY), dtype=np.float32)


# revision 10
# speedup vs baseline: 1.7529x; 1.0414x over previous
"""ConvexSH ColBERT loss kernel for 8 trn2 NeuronCores (v4).

Shards batch B=128 over 8 cores (16 rows each); host averages the 8 partial
sums. Doc layout per candidate n: one fully CONTIGUOUS 2 MB SWDGE cast-DMA
(f32->bf16) into [128, 32, 128] where partition p = b*8 + e covers tokens
k = e*32 + k'. Global k order is permuted - harmless under MaxSim's max.

Software-pipelined emission: iteration i emits square(i+1) | norm-chain(i) |
transposes+evac+matmul+max(i-1), so no engine's program order couples a
block's early stages to the previous block's late stages.

ssq = ACT Square -> two DVE pair-adds at 2x (TENSOR_REDUCE has no 2x mode,
tensor_tensor does) -> small 1x reduce. Normalize is one DVE tensor_tensor
with a pair-broadcast scale AP. Transposes/evacs go in 4 quarter-tiles with
a tunable ACT/DVE split. Label-only loss terms are precomputed on host; the
tail runs on flat [4,32] views with broadcast APs, and a manually emitted
InstLoadActFuncSet(natural_log_exp_and_others) after the last Sqrt hides
the tail's activation-table swap behind the final blocks.
"""

import sys
from contextlib import ExitStack

import numpy as np
import ml_dtypes

BF16NP = ml_dtypes.bfloat16

for _p in ("/opt/trn_rl_repo", "/root/.axon_site/_ro/trn_rl_repo"):
    if _p not in sys.path:
        sys.path.append(_p)

import concourse.bacc as bacc
import concourse.tile as tile
from concourse import mybir
from concourse.bass_utils import run_bass_kernel_spmd

AF = mybir.ActivationFunctionType
AX = mybir.AxisListType
ALU = mybir.AluOpType
F32 = mybir.dt.float32
BF16 = mybir.dt.bfloat16

NCORES = 8
B, LQ, LD, D, NWAY = 128, 32, 256, 128, 8
BS = B // NCORES  # 16 batch rows per core
NG = BS // 4      # 4 groups of 4 rows
NE = LD // 32     # 8 eighths of tokens per row -> partition p = b*8 + e
KT = 32           # tokens per partition (k')
ALPHA, GAMMA = 0.2, 2.0

EVAC_DVE_QUARTERS = 1   # how many of the 4 evac quarters go to DVE (rest ACT)
NLE_SET_ID = 6          # natural_log_exp_and_others in act_info.json

TRACE = False
LAST_RESULTS = None


def _build():
    nc = bacc.Bacc("TRN2", target_bir_lowering=False, detect_race_conditions=False)

    q_d = nc.dram_tensor("q", [128, NG, D], BF16, kind="ExternalInput")
    doc_d = nc.dram_tensor("doc", [NWAY, BS, LD, D], F32, kind="ExternalInput")
    mask_d = nc.dram_tensor("mask", [128, NWAY, KT], BF16, kind="ExternalInput")
    lab_d = nc.dram_tensor("lab", [BS, 6 * NWAY], F32, kind="ExternalInput")
    eye_d = nc.dram_tensor("eye", [128, 128], BF16, kind="ExternalInput")
    y_d = nc.dram_tensor("y", [1, 1], F32, kind="ExternalOutput")

    with tile.TileContext(nc) as tc, ExitStack() as ctx:
        singles = ctx.enter_context(tc.tile_pool(name="singles", bufs=1))
        dnp = ctx.enter_context(tc.tile_pool(name="dnp", bufs=6))
        sqp = ctx.enter_context(tc.tile_pool(name="sqp", bufs=3))
        dtp = ctx.enter_context(tc.tile_pool(name="dtp", bufs=2))
        smp = ctx.enter_context(tc.tile_pool(name="smp", bufs=3))
        psT = ctx.enter_context(tc.tile_pool(name="psT", bufs=4, space="PSUM"))
        psS = ctx.enter_context(tc.tile_pool(name="psS", bufs=2, space="PSUM"))

        # ---- constants -----------------------------------------------------
        eye_bf = singles.tile([128, 128], BF16)
        nc.sync.dma_start(out=eye_bf, in_=eye_d[:, :])

        blockones = singles.tile([128, NG], F32)
        nc.vector.memset(blockones, 0.0)
        for m in range(4):
            nc.vector.memset(blockones[m * 32:(m + 1) * 32, m:m + 1], 1.0)
        ones4 = singles.tile([4, 1], F32)
        nc.vector.memset(ones4, 1.0)

        # host-precomputed label constants: [t, a, b1, lnt, wts, w] x NWAY
        lab_sb = singles.tile([4, NG, 6 * NWAY], F32)
        nc.sync.dma_start(out=lab_sb, in_=lab_d.rearrange("(g m) c -> m g c", m=4))

        # masks in the (b, e) x (n, k') layout, cast to bf16
        mask_b = singles.tile([128, NWAY, KT], BF16)
        nc.sync.dma_start(out=mask_b, in_=mask_d[:, :, :])

        # ---- query path ----------------------------------------------------
        q_nat = singles.tile([128, NG, D], BF16)
        nc.sync.dma_start(out=q_nat, in_=q_d[:, :, :])

        ssq_q = singles.tile([128, NG], F32)
        qsq = singles.tile([128, D], BF16)
        for g in range(NG):
            nc.vector.scalar_tensor_tensor(
                out=qsq, in0=q_nat[:, g, :], scalar=1.0, in1=q_nat[:, g, :],
                op0=ALU.mult, op1=ALU.mult,
                accum_out=ssq_q[:, g:g + 1])
        invq = singles.tile([128, NG], F32)
        nc.scalar.activation(out=invq, in_=ssq_q, func=AF.Sqrt)
        nc.vector.reciprocal(invq, invq)

        qT = singles.tile([128, NG, 128], BF16)
        ps_q = psT.tile([128, 8, 128], BF16, tag="psT")
        for g in range(NG):
            nc.tensor.transpose(ps_q[:, g, :], q_nat[:, g, :], eye_bf)
        nc.scalar.copy(qT.rearrange("p a b -> p (a b)"),
                       ps_q[:, 0:NG, :].rearrange("p a b -> p (a b)"))

        maxs = singles.tile([128, NG, NWAY], F32)

        # ---- software-pipelined main loop ---------------------------------
        state = {}

        def stage_dma(n, halves=False):
            dn = dnp.tile([128, KT, D], BF16, tag="dn", name=f"dn{n}")
            src_ap = doc_d[n].rearrange("b (e t) d -> (b e) (t d)", e=NE)
            if halves:
                H = KT // 2 * D
                nc.gpsimd.dma_start(out=dn.rearrange("p t d -> p (t d)")[:, 0:H],
                                    in_=src_ap[:, 0:H])
                nc.gpsimd.dma_start(out=dn.rearrange("p t d -> p (t d)")[:, H:2 * H],
                                    in_=src_ap[:, H:2 * H])
            else:
                nc.gpsimd.dma_start(out=dn.rearrange("p t d -> p (t d)"), in_=src_ap)
            state[n] = {"dn": dn}

        def stage_square(n, halves=False):
            sq = sqp.tile([128, KT, D], BF16, tag="sq", name=f"sq{n}")
            dnf = state[n]["dn"].rearrange("p t d -> p (t d)")
            sqf = sq.rearrange("p t d -> p (t d)")
            if halves:
                H = KT // 2 * D
                nc.scalar.activation(out=sqf[:, 0:H], in_=dnf[:, 0:H], func=AF.Square)
                nc.scalar.activation(out=sqf[:, H:2 * H], in_=dnf[:, H:2 * H],
                                     func=AF.Square)
            else:
                nc.scalar.activation(out=sqf, in_=dnf, func=AF.Square)
            state[n]["sq"] = sq

        def stage_norm(n):
            dn, sq = state[n]["dn"], state[n]["sq"]
            # two pair-add stages at DVE 2x, then a small 1x reduce
            nc.vector.tensor_add(sq[:, :, 0:64], sq[:, :, 0:64], sq[:, :, 64:128])
            nc.vector.tensor_add(sq[:, :, 0:32], sq[:, :, 0:32], sq[:, :, 32:64])
            nc.vector.tensor_add(sq[:, :, 0:16], sq[:, :, 0:16], sq[:, :, 16:32])
            ssq = smp.tile([128, KT], F32, tag="ssq", name=f"ssq{n}")
            nc.vector.reduce_sum(out=ssq, in_=sq[:, :, 0:16], axis=AX.X)
            rt = smp.tile([128, KT], F32, tag="rt", name=f"rt{n}")
            nc.vector.reciprocal_approx_fast(rt, ssq)
            nc.scalar.activation(out=rt, in_=rt, func=AF.Sqrt)  # 1/||d||
            scale2 = smp.tile([128, KT, 2], BF16, tag="scale2", name=f"s2{n}")
            nc.vector.tensor_mul(scale2[:, :, 0], rt, mask_b[:, n, :])
            nc.scalar.copy(scale2[:, :, 1], scale2[:, :, 0])
            dn4 = dn.rearrange("p t (h w) -> p t h w", w=2)
            nc.vector.tensor_tensor(
                out=dn4, in0=dn4,
                in1=scale2.unsqueeze(2).broadcast_to([128, KT, D // 2, 2]),
                op=ALU.mult)

        def stage_sim(n):
            dn = state[n]["dn"]
            dT = dtp.tile([128, KT, 128], BF16, tag="dT", name=f"dT{n}")
            for qt in range(4):
                ps = psT.tile([128, 8, 128], BF16, tag="psT", name=f"ps{n}_{qt}")
                for j in range(8):
                    nc.tensor.transpose(ps[:, j, :], dn[:, qt * 8 + j, :], eye_bf)
                quarter = dT[:, qt * 8:(qt + 1) * 8, :]
                if qt < 4 - EVAC_DVE_QUARTERS:
                    nc.scalar.copy(quarter.rearrange("p t d -> p (t d)"),
                                   ps.rearrange("p t d -> p (t d)"))
                else:
                    nc.vector.tensor_copy(quarter.rearrange("p t d -> p (t d)"),
                                          ps.rearrange("p t d -> p (t d)"))
            sim = psS.tile([128, NG, 256], F32, tag="sim", name=f"sim{n}")
            for g in range(NG):
                for m in range(4):
                    b = g * 4 + m
                    nc.tensor.matmul(sim[m * 32:(m + 1) * 32, g, :],
                                     lhsT=qT[:, g, m * 32:(m + 1) * 32],
                                     rhs=dT[:, :, NE * b:NE * (b + 1)],
                                     start=True, stop=True,
                                     tile_position=(0, m * 32))
            nc.vector.reduce_max(out=maxs[:, :, n], in_=sim, axis=AX.X)
            nc.vector.tensor_mul(maxs[:, :, n], maxs[:, :, n], invq)
            del state[n]

        stage_dma(0, halves=True)
        stage_dma(1)
        stage_square(0, halves=True)
        for i in range(NWAY):
            if i + 2 < NWAY:
                stage_dma(i + 2)
            if i + 1 < NWAY:
                stage_square(i + 1)
            stage_norm(i)
            if i >= 1:
                stage_sim(i - 1)
        stage_sim(NWAY - 1)

        # ---- scores --------------------------------------------------------
        scores_ps = psT.tile([4, NG * NWAY], F32, tag="psT")
        nc.tensor.matmul(scores_ps, lhsT=blockones,
                         rhs=maxs.rearrange("p g n -> p (g n)"),
                         start=True, stop=True)
        sc = singles.tile([4, NG * NWAY], F32)  # [m, g*8+n]
        nc.vector.tensor_copy(sc, scores_ps)

        # ---- softmax over n (per g-slice); one Exp -------------------------
        sm = singles.tile([4, NG], F32)
        sc3 = sc.rearrange("p (g n) -> p g n", g=NG)
        nc.scalar.activation(out=sc, in_=sc, func=AF.Exp)
        nc.vector.reduce_sum(out=sm, in_=sc3, axis=AX.X)
        nc.vector.reciprocal(sm, sm)
        nc.vector.tensor_tensor(out=sc3, in0=sc3,
                                in1=sm.unsqueeze(2).broadcast_to([4, NG, NWAY]),
                                op=ALU.mult)

        # ---- ConvexSH loss (label-only terms precomputed on host) ---------
        F = NG * NWAY

        def fld(i):
            return lab_sb[:, :, i * NWAY:(i + 1) * NWAY]
        t3, a3, b13, lnt3, wts3, w3 = (fld(i) for i in range(6))

        def t32(name):
            t = singles.tile([4, F], F32, tag=name)
            return t, t.rearrange("p (g n) -> p g n", g=NG)

        # pom[:, 0, :] = p2, pom[:, 1, :] = 1 - p2; Ln and Exp run batched
        pom = singles.tile([4, 2, F], F32, tag="pom")
        pom4 = pom.rearrange("p a (g n) -> p a g n", g=NG)
        nc.vector.tensor_mul(pom4[:, 0], a3, sc3)
        nc.vector.tensor_add(pom4[:, 0], pom4[:, 0], b13)
        nc.vector.tensor_scalar(out=pom[:, 1, :], in0=pom[:, 0, :],
                                scalar1=-1.0, scalar2=1.0,
                                op0=ALU.mult, op1=ALU.add)
        pomf = pom.rearrange("p a f -> p (a f)")
        nc.scalar.activation(out=pomf, in_=pomf, func=AF.Ln)
        losses, losses3 = t32("losses")
        nc.vector.tensor_sub(losses3, lnt3, pom4[:, 0])
        nc.vector.tensor_mul(losses3, losses3, t3)
        nc.vector.tensor_tensor(
            out=pom4, in0=pom4,
            in1=wts3.unsqueeze(1).broadcast_to([4, 2, NG, NWAY]), op=ALU.mult)
        nc.scalar.activation(out=pomf, in_=pomf, func=AF.Exp)
        # pom[:, 0] = p2^wts, pom[:, 1] = (1-p2)^wts
        lv, lv3 = t32("lv")
        nc.vector.tensor_mul(lv3, w3, pom4[:, 1])
        t2, t23 = t32("t2")
        nc.vector.tensor_mul(t23, b13, pom4[:, 0])
        nc.vector.tensor_add(lv, lv, t2)
        nc.vector.tensor_mul(lv, lv, losses)

        partial = singles.tile([4, 1], F32)
        nc.vector.reduce_sum(out=partial, in_=lv, axis=AX.X)
        out_ps = psT.tile([1, 1], F32, tag="psT")
        nc.tensor.matmul(out_ps, lhsT=ones4, rhs=partial, start=True, stop=True)
        out_sb = singles.tile([1, 1], F32)
        nc.vector.tensor_copy(out_sb, out_ps)
        nc.sync.dma_start(out=y_d[:, :], in_=out_sb)

    nc.finalize()
    return nc


_nc_cache = None


def _q2(q):
    # [(m q), g, d] so the device upload is one contiguous 2 KB/partition DMA
    return np.ascontiguousarray(
        q.reshape(NG, 4, LQ, D).transpose(1, 2, 0, 3).reshape(128, NG, D))


def _m2(m):
    # [(b e), n, k'] matching the contiguous doc layout
    return np.ascontiguousarray(
        m.reshape(NWAY, BS, NE, KT).transpose(1, 2, 0, 3).reshape(128, NWAY, KT))


def _lab2(labels):
    t = labels[:, :NWAY].astype(np.float64)
    r = labels[:, NWAY:2 * NWAY].astype(np.float64)
    w = labels[:, 2 * NWAY:].astype(np.float64)
    a = 2.0 * w - 1.0
    b1 = 1.0 - w
    tinv = t * w + (1.0 - t) * (1.0 - w)
    lnt = np.log(tinv)
    rr = 1.0 / r
    wts = GAMMA - ALPHA * (rr - rr[:, :1])
    out = np.concatenate([t, a, b1, lnt, wts, w], axis=1)
    return np.ascontiguousarray(out, dtype=np.float32)


def kernel(query_reps, doc_reps, doc_masks, labels):
    global _nc_cache, LAST_RESULTS
    if _nc_cache is None:
        _nc_cache = _build()
    nc = _nc_cache

    eye = np.eye(128, dtype=BF16NP)
    labels = np.asarray(labels)
    in_maps = []
    for c in range(NCORES):
        sl = slice(c * BS, (c + 1) * BS)
        in_maps.append({
            "q": _q2(np.asarray(query_reps[sl], dtype=np.float32)).astype(BF16NP),
            "doc": np.ascontiguousarray(doc_reps[:, sl]).astype(np.float32, copy=False),
            "mask": _m2(np.asarray(doc_masks[:, sl], dtype=np.float32)).astype(BF16NP),
            "lab": _lab2(labels[sl]),
            "eye": eye,
        })

    kwargs = {}
    if TRACE:
        kwargs["trace"] = True
    res = run_bass_kernel_spmd(nc, in_maps, core_ids=list(range(NCORES)), **kwargs)
    LAST_RESULTS = res
    total = sum(float(res.results[c]["y"][0, 0]) for c in range(NCORES))
    return np.array(total / (B * NWAY), dtype=np.float32)
